# revision 1
# baseline (speedup 1.0000x reference)
"""Trainium2 Bass kernel for nn_Conv2d_86191403696259 (fp16 bands layout).

Originally: 1x HBM input read. Layout p = 32*dh + 3j + ic (K=88, zero-weight gaps).

Per chunk: DMA-load staging [24, (S+2)*WP] once; DVE-copy the three dh-shifted
views into 32-aligned partition groups of the slab; matmul as in v1.
PSUM->SBUF copies on ScalarE; per-s output DMAs.
"""

import ml_dtypes
import numpy as np

import concourse.bass as bass
import concourse.mybir as mybir
import concourse.tile as tile
from concourse import bacc
from concourse.bass_utils import run_bass_kernel_spmd

IC, OC, KH, KW = 3, 16, 3, 3
H = W = 2048
N_CORES = 8
RPC = H // N_CORES          # 256
HP = RPC + 2                # 258
WP = W + 2                  # 2050

NB = 8                      # bands
BR = RPC // NB              # 32 rows per band
S = 4                       # s-steps per chunk
NCHUNK = BR // S            # 8
NWT = W // 512              # 4
KP = 88                     # contraction partitions (with gaps)

F32 = mybir.dt.float32
F32R = mybir.dt.float32r
BF16 = mybir.dt.bfloat16
FP16 = mybir.dt.float16
DT = FP16


def build_nc() -> bass.Bass:
    nc = bacc.Bacc("TRN2", target_bir_lowering=False, debug=False)
    x = nc.dram_tensor("x", [IC, HP, WP], DT, kind="ExternalInput")
    wd = nc.dram_tensor("wd", [KW, KP, 128], DT, kind="ExternalInput")
    out = nc.dram_tensor("out", [OC, RPC, W], F32, kind="ExternalOutput")

    with tile.TileContext(nc) as tc:
        with (
            tc.tile_pool(name="wpool", bufs=1) as wpool,
            tc.tile_pool(name="slabp", bufs=1) as slab_pool,
            tc.tile_pool(name="stgin", bufs=2) as stgin_pool,
            tc.tile_pool(name="stgout", bufs=3) as stgout_pool,
            tc.tile_pool(name="psum", bufs=2, space="PSUM") as psum_pool,
        ):
            w_sb = wpool.tile([KP, KW * 128], DT)
            nc.sync.dma_start(out=w_sb[:, :], in_=wd.rearrange("dw p m -> p dw m"))

            # one persistent slab, two halves (chunk parity); zero the gap
            # partition groups once so matmul contraction reads finite zeros.
            slab = slab_pool.tile([KP, 2 * S * WP], DT)
            nc.vector.memset(slab[:, :], 0.0)

            for kc in range(NCHUNK):
                half = (kc % 2) * S * WP
                stin = stgin_pool.tile([24, (S + 2) * WP], DT, tag="stin")
                for u in range(S + 2):
                    rs = S * kc + u
                    src = x[:, rs : rs + (NB - 1) * BR + 1 : BR, :]
                    nc.sync.dma_start(
                        out=stin[:, u * WP : (u + 1) * WP],
                        in_=src.rearrange("ic j w -> j ic w"),
                    )
                for dh in range(KH):
                    nc.vector.tensor_copy(
                        out=slab[32 * dh : 32 * dh + 24, half : half + S * WP],
                        in_=stin[:, dh * WP : dh * WP + S * WP],
                    )

                for s in range(S):
                    ps = psum_pool.tile([128, W], F32, tag="ps")
                    for dw in range(KW):
                        for wt in range(NWT):
                            nc.tensor.matmul(
                                out=ps[:, wt * 512 : (wt + 1) * 512],
                                lhsT=w_sb[:, dw * 128 : (dw + 1) * 128],
                                rhs=slab[
                                    :,
                                    half + s * WP + wt * 512 + dw : half
                                    + s * WP
                                    + wt * 512
                                    + dw
                                    + 512,
                                ],
                                start=(dw == 0),
                                stop=(dw == KW - 1),
                            )
                    stg = stgout_pool.tile([128, W], F32, tag="stg")
                    nc.scalar.copy(out=stg[:, :], in_=ps[:, :])
                    rs = S * kc + s
                    dst = out[:, rs : rs + (NB - 1) * BR + 1 : BR, :]
                    # issue stores from ScalarE (HWDGE) so the Sync engine's
                    # FIFO only carries input loads and never blocks them
                    # behind store->copy->matmul dependency chains.
                    nc.scalar.dma_start(
                        out=dst.rearrange("oc j w -> j oc w"), in_=stg[:, :]
                    )

    nc.compile()
    return nc


def make_wdiag(kernel: np.ndarray) -> np.ndarray:
    """kernel [OC, IC, KH, KW] -> lhsT stack [KW, KP, 128], gaps zeroed."""
    wdg = np.zeros((KW, KP, 128), np.float32)
    for dw in range(KW):
        for dh in range(KH):
            for j in range(NB):
                for ic in range(IC):
                    wdg[dw, 32 * dh + 3 * j + ic, 16 * j : 16 * j + OC] = kernel[
                        :, ic, dh, dw
                    ]
    return wdg


_NC_CACHE = {}


def kernel(x: np.ndarray, kernel: np.ndarray) -> np.ndarray:
    assert x.shape == (IC, H, W) and kernel.shape == (OC, IC, KH, KW)
    x = np.ascontiguousarray(x, np.float32)
    kernel = np.ascontiguousarray(kernel, np.float32)

    if "nc" not in _NC_CACHE:
        _NC_CACHE["nc"] = build_nc()
    nc = _NC_CACHE["nc"]

    x_pad = np.zeros((IC, H + 2, W + 2), np.float16)
    x_pad[:, 1:-1, 1:-1] = x.astype(np.float16)
    wd = make_wdiag(kernel).astype(np.float16)

    in_maps = []
    for c in range(N_CORES):
        slab = np.ascontiguousarray(x_pad[:, c * RPC : c * RPC + HP, :])
        in_maps.append({"x": slab, "wd": wd})

    res = run_bass_kernel_spmd(nc, in_maps, core_ids=list(range(N_CORES)))
    outs = [res.results[c]["out"] for c in range(N_CORES)]
    return np.concatenate(outs, axis=1)



# revision 8
# speedup vs baseline: 2.4645x; 2.4645x over previous
"""Trainium2 Bass kernel for nn_Conv2d_86191403696259 (single-pass conv).

Layout: consecutive-row bands. M partitions = 16 OC x 8 consecutive rows;
K partitions = (dw, input row rl 0..9, ic) -> 32*dw + 3*rl + ic (96 with gaps).
dh is folded into the band weight matrix (consecutive rows share input
partitions), so each output column needs ONE matmul pass (vs 3 for a
dw-accumulation loop) and the lhsT is stationary across the whole kernel.

dw replication: dw=0 and dw=2 slab groups are 4-byte-aligned shifted copies
of staging0 (DVE int32 2x copies); the odd 2-byte dw=1 shift is loaded
directly from HBM as a second staging read (byte-addressed DMA).

Input is pre-grouped on the host into 32-aligned per-step partition groups
(engine APs must start at 32-aligned partitions):
  xg[128*kc + 32*s + 3*rl + ic, w] = x_pad[ic, 32*kc + 8*s + rl, w]
Output is written fp16 (rel-err budget 2e-2 >> fp16 quantization) in a
partition-major layout (contiguous 16KB per partition per store) and
reassembled + upcast to f32 on the host, halving HBM store traffic.
"""

import ml_dtypes
import numpy as np

import concourse.bass as bass
import concourse.mybir as mybir
import concourse.tile as tile
from concourse import bacc
from concourse.bass_utils import run_bass_kernel_spmd

IC, OC, KH, KW = 3, 16, 3, 3
H = W = 2048
N_CORES = 8
RPC = H // N_CORES          # 256 output rows per core
HP = RPC + 2                # 258 padded input rows per core
WP = W + 2                  # 2050
S = 4                       # s-steps per chunk (8 rows each)
NCHUNK = RPC // (8 * S)     # 8
NWT = W // 512              # 4
KP = 96                     # contraction partitions (with gaps)

F32 = mybir.dt.float32
FP16 = mybir.dt.float16
U32 = mybir.dt.uint32
DT = FP16
NPDT = np.float16


def build_nc() -> bass.Bass:
    nc = bacc.Bacc("TRN2", target_bir_lowering=False, debug=False)
    xg = nc.dram_tensor("xg", [128 * NCHUNK, WP], DT, kind="ExternalInput")
    wd = nc.dram_tensor("wd", [KP, 128], DT, kind="ExternalInput")
    # blk[p, sb, w] with p = 16*j + oc, row = 8*sb + j
    blk = nc.dram_tensor("blk", [128, RPC // 8, W], DT, kind="ExternalOutput")

    with tile.TileContext(nc) as tc:
        with (
            tc.tile_pool(name="wpool", bufs=1) as wpool,
            tc.tile_pool(name="slabp", bufs=1) as slab_pool,
            tc.tile_pool(name="stg0", bufs=2) as stg0_pool,
            tc.tile_pool(name="stg1", bufs=2) as stg1_pool,
            tc.tile_pool(name="stgout", bufs=2) as stgout_pool,
            tc.tile_pool(name="psum", bufs=2, space="PSUM") as psum_pool,
        ):
            w_sb = wpool.tile([KP, 128], DT)
            nc.sync.dma_start(out=w_sb[:, :], in_=wd[:, :])

            # persistent slab, two halves (step parity); zero once so gap
            # partitions read as finite zeros under the zero weights.
            slab = slab_pool.tile([KP, 2 * W], DT)
            nc.vector.memset(slab[:, :], 0.0)

            for kc in range(NCHUNK):
                st0 = stg0_pool.tile([128, WP], DT, tag="st0")
                nc.sync.dma_start(
                    out=st0[:, :], in_=xg[128 * kc : 128 * kc + 128, :]
                )
                st1 = stg1_pool.tile([128, W], DT, tag="st1")
                nc.sync.dma_start(
                    out=st1[:, :], in_=xg[128 * kc : 128 * kc + 128, 1 : 1 + W]
                )
                stg = stgout_pool.tile([128, S * W], DT, tag="stg")
                for s in range(S):
                    h = ((kc * S + s) % 2) * W
                    q = 32 * s
                    nc.vector.tensor_copy(
                        out=slab[0:30, h : h + W].bitcast(U32),
                        in_=st0[q : q + 30, 0:W].bitcast(U32),
                    )
                    nc.vector.tensor_copy(
                        out=slab[32:62, h : h + W].bitcast(U32),
                        in_=st1[q : q + 30, 0:W].bitcast(U32),
                    )
                    nc.vector.tensor_copy(
                        out=slab[64:94, h : h + W].bitcast(U32),
                        in_=st0[q : q + 30, 2 : 2 + W].bitcast(U32),
                    )
                    ps = psum_pool.tile([128, W], F32, tag="ps")
                    for wt in range(NWT):
                        nc.tensor.matmul(
                            out=ps[:, wt * 512 : (wt + 1) * 512],
                            lhsT=w_sb[:, :],
                            rhs=slab[:, h + wt * 512 : h + wt * 512 + 512],
                            start=True,
                            stop=True,
                        )
                    nc.scalar.copy(out=stg[:, s * W : (s + 1) * W], in_=ps[:, :])
                # store 32 rows (4 steps) in one 2MB DMA; issued from ScalarE
                # (HWDGE) so the Sync queue only carries input loads.
                nc.scalar.dma_start(
                    out=blk[:, S * kc : S * kc + S, :], in_=stg[:, :]
                )

    nc.compile()
    return nc


def make_wdiag(kernel: np.ndarray) -> np.ndarray:
    """kernel [OC, IC, KH, KW] -> stationary lhsT [KP, 128], gaps zeroed."""
    wdg = np.zeros((KP, 128), np.float32)
    for dw in range(KW):
        for j in range(8):
            for dh in range(KH):
                rl = j + dh
                for ic in range(IC):
                    wdg[32 * dw + 3 * rl + ic, 16 * j : 16 * j + OC] = kernel[
                        :, ic, dh, dw
                    ]
    return wdg


def prepare_in_maps(x: np.ndarray, kernel: np.ndarray) -> list:
    x_pad = np.zeros((IC, H + 2, W + 2), NPDT)
    x_pad[:, 1:-1, 1:-1] = x.astype(NPDT)
    wd = make_wdiag(kernel).astype(NPDT)
    # row index per (kc, s, rl): 32*kc + 8*s + rl
    rows = (
        32 * np.arange(NCHUNK)[:, None, None]
        + 8 * np.arange(S)[None, :, None]
        + np.arange(10)[None, None, :]
    )  # [NCHUNK, S, 10]
    in_maps = []
    for c in range(N_CORES):
        slab = x_pad[:, c * RPC : c * RPC + HP, :]          # [IC, HP, WP]
        g = slab[:, rows, :]                                # [IC, NCHUNK, S, 10, WP]
        g = g.transpose(1, 2, 3, 0, 4)                      # [NCHUNK, S, 10, IC, WP]
        xg = np.zeros((NCHUNK, S, 32, WP), NPDT)
        xg[:, :, :30, :] = g.reshape(NCHUNK, S, 30, WP)
        in_maps.append({"xg": xg.reshape(128 * NCHUNK, WP), "wd": wd})
    return in_maps


def gather_out(blk: np.ndarray) -> np.ndarray:
    """blk [128, RPC//8, W] (p = 16j+oc, row = 8sb+j) -> [OC, RPC, W]."""
    t = blk.reshape(8, 16, RPC // 8, W).transpose(1, 2, 0, 3)
    return t.reshape(OC, RPC, W)


_NC_CACHE = {}


def kernel(x: np.ndarray, kernel: np.ndarray) -> np.ndarray:
    assert x.shape == (IC, H, W) and kernel.shape == (OC, IC, KH, KW)
    x = np.ascontiguousarray(x, np.float32)
    kernel = np.ascontiguousarray(kernel, np.float32)

    if "nc" not in _NC_CACHE:
        _NC_CACHE["nc"] = build_nc()
    nc = _NC_CACHE["nc"]

    in_maps = prepare_in_maps(x, kernel)
    res = run_bass_kernel_spmd(nc, in_maps, core_ids=list(range(N_CORES)))
    outs = [gather_out(res.results[c]["blk"]) for c in range(N_CORES)]
    return np.concatenate(outs, axis=1).astype(np.float32)


# revision 9
# speedup vs baseline: 2.4830x; 1.0075x over previous
"""Trainium2 Bass kernel for nn_Conv2d_86191403696259 (single-pass conv).

Layout: consecutive-row bands. M partitions = 16 OC x 8 consecutive rows;
K partitions = (dw, input row rl 0..9, ic) -> 32*dw + 3*rl + ic (96 with gaps).
dh is folded into the band weight matrix (consecutive rows share input
partitions), so each output column needs ONE matmul pass (vs 3 for a
dw-accumulation loop) and the lhsT is stationary across the whole kernel.

dw replication: dw=0 and dw=2 slab groups are 4-byte-aligned shifted copies
of staging0 (DVE int32 2x copies); the odd 2-byte dw=1 shift is loaded
directly from HBM as a second staging read (byte-addressed DMA).

Input is pre-grouped on the host into 32-aligned per-step partition groups
(engine APs must start at 32-aligned partitions):
  xg[128*kc + 32*s + 3*rl + ic, w] = x_pad[ic, 32*kc + 8*s + rl, w]
Output is written fp16 (rel-err budget 2e-2 >> fp16 quantization) in a
partition-major layout (contiguous 16KB per partition per store) and
reassembled + upcast to f32 on the host, halving HBM store traffic.
"""

import ml_dtypes
import numpy as np

import concourse.bass as bass
import concourse.mybir as mybir
import concourse.tile as tile
from concourse import bacc
from concourse.bass_utils import run_bass_kernel_spmd

IC, OC, KH, KW = 3, 16, 3, 3
H = W = 2048
N_CORES = 8
RPC = H // N_CORES          # 256 output rows per core
HP = RPC + 2                # 258 padded input rows per core
WP = W + 2                  # 2050
S = 4                       # s-steps per chunk (8 rows each)
NCHUNK = RPC // (8 * S)     # 8
NWT = W // 512              # 4
KP = 96                     # contraction partitions (with gaps)

F32 = mybir.dt.float32
FP16 = mybir.dt.float16
U32 = mybir.dt.uint32
DT = FP16
NPDT = np.float16


def build_nc() -> bass.Bass:
    nc = bacc.Bacc("TRN2", target_bir_lowering=False, debug=False)
    xg = nc.dram_tensor("xg", [128 * NCHUNK, WP], DT, kind="ExternalInput")
    wd = nc.dram_tensor("wd", [KP, 128], DT, kind="ExternalInput")
    # blk[p, sb, w] with p = 16*j + oc, row = 8*sb + j
    blk = nc.dram_tensor("blk", [128, RPC // 8, W], DT, kind="ExternalOutput")

    with tile.TileContext(nc) as tc:
        with (
            tc.tile_pool(name="wpool", bufs=1) as wpool,
            tc.tile_pool(name="slabp", bufs=1) as slab_pool,
            tc.tile_pool(name="stg0", bufs=3) as stg0_pool,
            tc.tile_pool(name="stg1", bufs=3) as stg1_pool,
            tc.tile_pool(name="stgout", bufs=2) as stgout_pool,
            tc.tile_pool(name="psum", bufs=2, space="PSUM") as psum_pool,
        ):
            w_sb = wpool.tile([KP, 128], DT)
            nc.sync.dma_start(out=w_sb[:, :], in_=wd[:, :])

            # persistent slab, two halves (step parity); zero once so gap
            # partitions read as finite zeros under the zero weights.
            slab = slab_pool.tile([KP, 2 * W], DT)
            nc.vector.memset(slab[:, :], 0.0)

            # one-time probe ops in the startup shadow (results overwritten by
            # the first real copies): measure DVE odd-offset copy mode and
            # ScalarE fp16 SBUF copy rate from their trace durations.
            nc.vector.tensor_copy(out=slab[64:94, 0:W], in_=slab[0:30, 1 : 1 + W])
            nc.scalar.copy(out=slab[32:62, 0:W], in_=slab[0:30, 0:W])

            for kc in range(NCHUNK):
                st0 = stg0_pool.tile([128, WP], DT, tag="st0")
                nc.sync.dma_start(
                    out=st0[:, :], in_=xg[128 * kc : 128 * kc + 128, :]
                )
                st1 = stg1_pool.tile([128, W], DT, tag="st1")
                nc.sync.dma_start(
                    out=st1[:, :], in_=xg[128 * kc : 128 * kc + 128, 1 : 1 + W]
                )
                stg = stgout_pool.tile([128, S * W], DT, tag="stg")
                for s in range(S):
                    h = ((kc * S + s) % 2) * W
                    q = 32 * s
                    nc.vector.tensor_copy(
                        out=slab[0:30, h : h + W].bitcast(U32),
                        in_=st0[q : q + 30, 0:W].bitcast(U32),
                    )
                    nc.vector.tensor_copy(
                        out=slab[32:62, h : h + W].bitcast(U32),
                        in_=st1[q : q + 30, 0:W].bitcast(U32),
                    )
                    nc.vector.tensor_copy(
                        out=slab[64:94, h : h + W].bitcast(U32),
                        in_=st0[q : q + 30, 2 : 2 + W].bitcast(U32),
                    )
                    ps = psum_pool.tile([128, W], F32, tag="ps")
                    for wt in range(NWT):
                        nc.tensor.matmul(
                            out=ps[:, wt * 512 : (wt + 1) * 512],
                            lhsT=w_sb[:, :],
                            rhs=slab[:, h + wt * 512 : h + wt * 512 + 512],
                            start=True,
                            stop=True,
                        )
                    nc.scalar.copy(out=stg[:, s * W : (s + 1) * W], in_=ps[:, :])
                if kc < NCHUNK - 1:
                    # store 32 rows (4 steps) in one 2MB DMA; issued from
                    # ScalarE (HWDGE) so the Sync queue only carries loads.
                    nc.scalar.dma_start(
                        out=blk[:, S * kc : S * kc + S, :], in_=stg[:, :]
                    )
                else:
                    # last chunk: store per step so the final drain overlaps
                    # the remaining compute instead of waiting for all 4 evacs
                    for s in range(S):
                        nc.scalar.dma_start(
                            out=blk[:, S * kc + s : S * kc + s + 1, :],
                            in_=stg[:, s * W : (s + 1) * W],
                        )

    nc.compile()
    return nc


def make_wdiag(kernel: np.ndarray) -> np.ndarray:
    """kernel [OC, IC, KH, KW] -> stationary lhsT [KP, 128], gaps zeroed."""
    wdg = np.zeros((KP, 128), np.float32)
    for dw in range(KW):
        for j in range(8):
            for dh in range(KH):
                rl = j + dh
                for ic in range(IC):
                    wdg[32 * dw + 3 * rl + ic, 16 * j : 16 * j + OC] = kernel[
                        :, ic, dh, dw
                    ]
    return wdg


def prepare_in_maps(x: np.ndarray, kernel: np.ndarray) -> list:
    x_pad = np.zeros((IC, H + 2, W + 2), NPDT)
    x_pad[:, 1:-1, 1:-1] = x.astype(NPDT)
    wd = make_wdiag(kernel).astype(NPDT)
    # row index per (kc, s, rl): 32*kc + 8*s + rl
    rows = (
        32 * np.arange(NCHUNK)[:, None, None]
        + 8 * np.arange(S)[None, :, None]
        + np.arange(10)[None, None, :]
    )  # [NCHUNK, S, 10]
    in_maps = []
    for c in range(N_CORES):
        slab = x_pad[:, c * RPC : c * RPC + HP, :]          # [IC, HP, WP]
        g = slab[:, rows, :]                                # [IC, NCHUNK, S, 10, WP]
        g = g.transpose(1, 2, 3, 0, 4)                      # [NCHUNK, S, 10, IC, WP]
        xg = np.zeros((NCHUNK, S, 32, WP), NPDT)
        xg[:, :, :30, :] = g.reshape(NCHUNK, S, 30, WP)
        in_maps.append({"xg": xg.reshape(128 * NCHUNK, WP), "wd": wd})
    return in_maps


def gather_out(blk: np.ndarray) -> np.ndarray:
    """blk [128, RPC//8, W] (p = 16j+oc, row = 8sb+j) -> [OC, RPC, W]."""
    t = blk.reshape(8, 16, RPC // 8, W).transpose(1, 2, 0, 3)
    return t.reshape(OC, RPC, W)


_NC_CACHE = {}


def kernel(x: np.ndarray, kernel: np.ndarray) -> np.ndarray:
    assert x.shape == (IC, H, W) and kernel.shape == (OC, IC, KH, KW)
    x = np.ascontiguousarray(x, np.float32)
    kernel = np.ascontiguousarray(kernel, np.float32)

    if "nc" not in _NC_CACHE:
        _NC_CACHE["nc"] = build_nc()
    nc = _NC_CACHE["nc"]

    in_maps = prepare_in_maps(x, kernel)
    res = run_bass_kernel_spmd(nc, in_maps, core_ids=list(range(N_CORES)))
    outs = [gather_out(res.results[c]["blk"]) for c in range(N_CORES)]
    return np.concatenate(outs, axis=1).astype(np.float32)


# revision 10
# speedup vs baseline: 2.5674x; 1.0340x over previous
"""Trainium2 Bass kernel for nn_Conv2d_86191403696259 (single-pass conv).

Layout: consecutive-row bands. M partitions = 16 OC x 8 consecutive rows;
K partitions = (dw, input row rl 0..9, ic) -> 32*dw + 3*rl + ic (96 with gaps).
dh is folded into the band weight matrix (consecutive rows share input
partitions), so each output column needs ONE matmul pass (vs 3 for a
dw-accumulation loop) and the lhsT is stationary across the whole kernel.

dw replication: all three dw slab groups are DVE shifted copies of the
single staging read (measured: DVE fp16 copies run at full 4x rate even at
odd 2-byte source offsets, so no second HBM read is needed).

Input is pre-grouped on the host into 32-aligned per-step partition groups
(engine APs must start at 32-aligned partitions):
  xg[128*kc + 32*s + 3*rl + ic, w] = x_pad[ic, 32*kc + 8*s + rl, w]
Output is written fp16 (rel-err budget 2e-2 >> fp16 quantization) in a
partition-major layout (contiguous 16KB per partition per store) and
reassembled + upcast to f32 on the host, halving HBM store traffic.
"""

import ml_dtypes
import numpy as np

import concourse.bass as bass
import concourse.mybir as mybir
import concourse.tile as tile
from concourse import bacc
from concourse.bass_utils import run_bass_kernel_spmd

IC, OC, KH, KW = 3, 16, 3, 3
H = W = 2048
N_CORES = 8
RPC = H // N_CORES          # 256 output rows per core
HP = RPC + 2                # 258 padded input rows per core
WP = W + 2                  # 2050
S = 4                       # s-steps per chunk (8 rows each)
NCHUNK = RPC // (8 * S)     # 8
NWT = W // 512              # 4
KP = 96                     # contraction partitions (with gaps)

F32 = mybir.dt.float32
FP16 = mybir.dt.float16
U32 = mybir.dt.uint32
DT = FP16
NPDT = np.float16


def build_nc() -> bass.Bass:
    nc = bacc.Bacc("TRN2", target_bir_lowering=False, debug=False)
    xg = nc.dram_tensor("xg", [128 * NCHUNK, WP], DT, kind="ExternalInput")
    wd = nc.dram_tensor("wd", [KP, 128], DT, kind="ExternalInput")
    # blk[p, sb, w] with p = 16*j + oc, row = 8*sb + j
    blk = nc.dram_tensor("blk", [128, RPC // 8, W], DT, kind="ExternalOutput")

    with tile.TileContext(nc) as tc:
        with (
            tc.tile_pool(name="wpool", bufs=1) as wpool,
            tc.tile_pool(name="slabp", bufs=1) as slab_pool,
            tc.tile_pool(name="stg0", bufs=3) as stg0_pool,
            tc.tile_pool(name="stgout", bufs=2) as stgout_pool,
            tc.tile_pool(name="psum", bufs=2, space="PSUM") as psum_pool,
        ):
            w_sb = wpool.tile([KP, 128], DT)
            nc.sync.dma_start(out=w_sb[:, :], in_=wd[:, :])

            # persistent slab, two halves (step parity); zero once so gap
            # partitions read as finite zeros under the zero weights.
            slab = slab_pool.tile([KP, 2 * W], DT)
            nc.vector.memset(slab[:, :], 0.0)

            for kc in range(NCHUNK):
                st0 = stg0_pool.tile([128, WP], DT, tag="st0")
                nc.sync.dma_start(
                    out=st0[:, :], in_=xg[128 * kc : 128 * kc + 128, :]
                )
                stg = stgout_pool.tile([128, S * W], DT, tag="stg")
                for s in range(S):
                    h = ((kc * S + s) % 2) * W
                    q = 32 * s
                    nc.vector.tensor_copy(
                        out=slab[0:30, h : h + W].bitcast(U32),
                        in_=st0[q : q + 30, 0:W].bitcast(U32),
                    )
                    nc.vector.tensor_copy(
                        out=slab[32:62, h : h + W],
                        in_=st0[q : q + 30, 1 : 1 + W],
                    )
                    nc.vector.tensor_copy(
                        out=slab[64:94, h : h + W].bitcast(U32),
                        in_=st0[q : q + 30, 2 : 2 + W].bitcast(U32),
                    )
                    ps = psum_pool.tile([128, W], F32, tag="ps")
                    for wt in range(NWT):
                        nc.tensor.matmul(
                            out=ps[:, wt * 512 : (wt + 1) * 512],
                            lhsT=w_sb[:, :],
                            rhs=slab[:, h + wt * 512 : h + wt * 512 + 512],
                            start=True,
                            stop=True,
                        )
                    nc.scalar.copy(out=stg[:, s * W : (s + 1) * W], in_=ps[:, :])
                if kc < NCHUNK - 1:
                    # store 32 rows (4 steps) in one 2MB DMA; issued from
                    # ScalarE (HWDGE) so the Sync queue only carries loads.
                    nc.scalar.dma_start(
                        out=blk[:, S * kc : S * kc + S, :], in_=stg[:, :]
                    )
                else:
                    # last chunk: store per step so the final drain overlaps
                    # the remaining compute instead of waiting for all 4 evacs
                    for s in range(S):
                        nc.scalar.dma_start(
                            out=blk[:, S * kc + s : S * kc + s + 1, :],
                            in_=stg[:, s * W : (s + 1) * W],
                        )

    nc.compile()
    return nc


def make_wdiag(kernel: np.ndarray) -> np.ndarray:
    """kernel [OC, IC, KH, KW] -> stationary lhsT [KP, 128], gaps zeroed."""
    wdg = np.zeros((KP, 128), np.float32)
    for dw in range(KW):
        for j in range(8):
            for dh in range(KH):
                rl = j + dh
                for ic in range(IC):
                    wdg[32 * dw + 3 * rl + ic, 16 * j : 16 * j + OC] = kernel[
                        :, ic, dh, dw
                    ]
    return wdg


def prepare_in_maps(x: np.ndarray, kernel: np.ndarray) -> list:
    x_pad = np.zeros((IC, H + 2, W + 2), NPDT)
    x_pad[:, 1:-1, 1:-1] = x.astype(NPDT)
    wd = make_wdiag(kernel).astype(NPDT)
    # row index per (kc, s, rl): 32*kc + 8*s + rl
    rows = (
        32 * np.arange(NCHUNK)[:, None, None]
        + 8 * np.arange(S)[None, :, None]
        + np.arange(10)[None, None, :]
    )  # [NCHUNK, S, 10]
    in_maps = []
    for c in range(N_CORES):
        slab = x_pad[:, c * RPC : c * RPC + HP, :]          # [IC, HP, WP]
        g = slab[:, rows, :]                                # [IC, NCHUNK, S, 10, WP]
        g = g.transpose(1, 2, 3, 0, 4)                      # [NCHUNK, S, 10, IC, WP]
        xg = np.zeros((NCHUNK, S, 32, WP), NPDT)
        xg[:, :, :30, :] = g.reshape(NCHUNK, S, 30, WP)
        in_maps.append({"xg": xg.reshape(128 * NCHUNK, WP), "wd": wd})
    return in_maps


def gather_out(blk: np.ndarray) -> np.ndarray:
    """blk [128, RPC//8, W] (p = 16j+oc, row = 8sb+j) -> [OC, RPC, W]."""
    t = blk.reshape(8, 16, RPC // 8, W).transpose(1, 2, 0, 3)
    return t.reshape(OC, RPC, W)


_NC_CACHE = {}


def kernel(x: np.ndarray, kernel: np.ndarray) -> np.ndarray:
    assert x.shape == (IC, H, W) and kernel.shape == (OC, IC, KH, KW)
    x = np.ascontiguousarray(x, np.float32)
    kernel = np.ascontiguousarray(kernel, np.float32)

    if "nc" not in _NC_CACHE:
        _NC_CACHE["nc"] = build_nc()
    nc = _NC_CACHE["nc"]

    in_maps = prepare_in_maps(x, kernel)
    res = run_bass_kernel_spmd(nc, in_maps, core_ids=list(range(N_CORES)))
    outs = [gather_out(res.results[c]["blk"]) for c in range(N_CORES)]
    return np.concatenate(outs, axis=1).astype(np.float32)


# revision 13
# speedup vs baseline: 2.5701x; 1.0010x over previous
"""Trainium2 Bass kernel for nn_Conv2d_86191403696259 (single-pass conv).

Layout: consecutive-row bands. M partitions = 16 OC x 8 consecutive rows;
K partitions = (dw, input row rl 0..9, ic) -> 32*dw + 3*rl + ic (96 with gaps).
dh is folded into the band weight matrix (consecutive rows share input
partitions), so each output column needs ONE matmul pass (vs 3 for a
dw-accumulation loop) and the lhsT is stationary across the whole kernel.

dw replication: all three dw slab groups are DVE shifted copies of the
single staging read (measured: DVE fp16 copies run at full 4x rate even at
odd 2-byte source offsets, so no second HBM read is needed).

Input is pre-grouped on the host into 32-aligned per-step partition groups
(engine APs must start at 32-aligned partitions):
  xg[128*kc + 32*s + 3*rl + ic, w] = x_pad[ic, 32*kc + 8*s + rl, w]
Output is written fp16 (rel-err budget 2e-2 >> fp16 quantization) in a
partition-major layout (contiguous 16KB per partition per store) and
reassembled + upcast to f32 on the host, halving HBM store traffic.
"""

import ml_dtypes
import numpy as np

import concourse.bass as bass
import concourse.mybir as mybir
import concourse.tile as tile
from concourse import bacc
from concourse.bass_utils import run_bass_kernel_spmd

IC, OC, KH, KW = 3, 16, 3, 3
H = W = 2048
N_CORES = 8
RPC = H // N_CORES          # 256 output rows per core
HP = RPC + 2                # 258 padded input rows per core
WP = W + 2                  # 2050
S = 4                       # s-steps per chunk (8 rows each)
NCHUNK = RPC // (8 * S)     # 8
NWT = W // 512              # 4
KP = 96                     # contraction partitions (with gaps)

F32 = mybir.dt.float32
FP16 = mybir.dt.float16
U32 = mybir.dt.uint32
DT = FP16
NPDT = np.float16


def build_nc() -> bass.Bass:
    nc = bacc.Bacc("TRN2", target_bir_lowering=False, debug=False)
    xg = nc.dram_tensor("xg", [128 * NCHUNK, WP], DT, kind="ExternalInput")
    wd = nc.dram_tensor("wd", [KP, 128], DT, kind="ExternalInput")
    # blk[p, sb, w] with p = 16*j + oc, row = 8*sb + j
    blk = nc.dram_tensor("blk", [128, RPC // 8, W], DT, kind="ExternalOutput")

    with tile.TileContext(nc) as tc:
        with (
            tc.tile_pool(name="wpool", bufs=1) as wpool,
            tc.tile_pool(name="slabp", bufs=1) as slab_pool,
            tc.tile_pool(name="stg0", bufs=3) as stg0_pool,
            tc.tile_pool(name="stgout", bufs=2) as stgout_pool,
            tc.tile_pool(name="psum", bufs=2, space="PSUM") as psum_pool,
        ):
            w_sb = wpool.tile([KP, 128], DT)
            nc.sync.dma_start(out=w_sb[:, :], in_=wd[:, :])

            # persistent slab, two halves (step parity); zero once so gap
            # partitions read as finite zeros under the zero weights.
            slab = slab_pool.tile([KP, 2 * W], DT)
            nc.vector.memset(slab[:, :], 0.0)

            for kc in range(NCHUNK):
                st0 = stg0_pool.tile([128, WP], DT, tag="st0")
                if kc == 0:
                    # split the first load so step-0 copies start as soon as
                    # the first 32-partition group lands (shorter pipe fill)
                    for g in range(4):
                        nc.sync.dma_start(
                            out=st0[32 * g : 32 * g + 32, :],
                            in_=xg[32 * g : 32 * g + 32, :],
                        )
                else:
                    nc.sync.dma_start(
                        out=st0[:, :], in_=xg[128 * kc : 128 * kc + 128, :]
                    )
                stg = stgout_pool.tile([128, S * W], DT, tag="stg")
                for s in range(S):
                    h = ((kc * S + s) % 2) * W
                    q = 32 * s
                    nc.vector.tensor_copy(
                        out=slab[0:30, h : h + W].bitcast(U32),
                        in_=st0[q : q + 30, 0:W].bitcast(U32),
                    )
                    nc.vector.tensor_copy(
                        out=slab[32:62, h : h + W],
                        in_=st0[q : q + 30, 1 : 1 + W],
                    )
                    nc.vector.tensor_copy(
                        out=slab[64:94, h : h + W].bitcast(U32),
                        in_=st0[q : q + 30, 2 : 2 + W].bitcast(U32),
                    )
                    ps = psum_pool.tile([128, W], F32, tag="ps")
                    for wt in range(NWT):
                        nc.tensor.matmul(
                            out=ps[:, wt * 512 : (wt + 1) * 512],
                            lhsT=w_sb[:, :],
                            rhs=slab[:, h + wt * 512 : h + wt * 512 + 512],
                            start=True,
                            stop=True,
                        )
                    nc.scalar.copy(out=stg[:, s * W : (s + 1) * W], in_=ps[:, :])
                if kc < NCHUNK - 1:
                    # store 32 rows (4 steps) in one 2MB DMA; issued from
                    # ScalarE (HWDGE) so the Sync queue only carries loads.
                    nc.scalar.dma_start(
                        out=blk[:, S * kc : S * kc + S, :], in_=stg[:, :]
                    )
                else:
                    # last chunk: store per step so the final drain overlaps
                    # the remaining compute instead of waiting for all 4 evacs
                    for s in range(S):
                        nc.scalar.dma_start(
                            out=blk[:, S * kc + s : S * kc + s + 1, :],
                            in_=stg[:, s * W : (s + 1) * W],
                        )

    nc.compile()
    return nc


def make_wdiag(kernel: np.ndarray) -> np.ndarray:
    """kernel [OC, IC, KH, KW] -> stationary lhsT [KP, 128], gaps zeroed."""
    wdg = np.zeros((KP, 128), np.float32)
    for dw in range(KW):
        for j in range(8):
            for dh in range(KH):
                rl = j + dh
                for ic in range(IC):
                    wdg[32 * dw + 3 * rl + ic, 16 * j : 16 * j + OC] = kernel[
                        :, ic, dh, dw
                    ]
    return wdg


def prepare_in_maps(x: np.ndarray, kernel: np.ndarray) -> list:
    x_pad = np.zeros((IC, H + 2, W + 2), NPDT)
    x_pad[:, 1:-1, 1:-1] = x.astype(NPDT)
    wd = make_wdiag(kernel).astype(NPDT)
    # row index per (kc, s, rl): 32*kc + 8*s + rl
    rows = (
        32 * np.arange(NCHUNK)[:, None, None]
        + 8 * np.arange(S)[None, :, None]
        + np.arange(10)[None, None, :]
    )  # [NCHUNK, S, 10]
    in_maps = []
    for c in range(N_CORES):
        slab = x_pad[:, c * RPC : c * RPC + HP, :]          # [IC, HP, WP]
        g = slab[:, rows, :]                                # [IC, NCHUNK, S, 10, WP]
        g = g.transpose(1, 2, 3, 0, 4)                      # [NCHUNK, S, 10, IC, WP]
        xg = np.zeros((NCHUNK, S, 32, WP), NPDT)
        xg[:, :, :30, :] = g.reshape(NCHUNK, S, 30, WP)
        in_maps.append({"xg": xg.reshape(128 * NCHUNK, WP), "wd": wd})
    return in_maps


def gather_out(blk: np.ndarray) -> np.ndarray:
    """blk [128, RPC//8, W] (p = 16j+oc, row = 8sb+j) -> [OC, RPC, W]."""
    t = blk.reshape(8, 16, RPC // 8, W).transpose(1, 2, 0, 3)
    return t.reshape(OC, RPC, W)


_NC_CACHE = {}


def kernel(x: np.ndarray, kernel: np.ndarray) -> np.ndarray:
    assert x.shape == (IC, H, W) and kernel.shape == (OC, IC, KH, KW)
    x = np.ascontiguousarray(x, np.float32)
    kernel = np.ascontiguousarray(kernel, np.float32)

    if "nc" not in _NC_CACHE:
        _NC_CACHE["nc"] = build_nc()
    nc = _NC_CACHE["nc"]

    in_maps = prepare_in_maps(x, kernel)
    res = run_bass_kernel_spmd(nc, in_maps, core_ids=list(range(N_CORES)))
    outs = [gather_out(res.results[c]["blk"]) for c in range(N_CORES)]
    return np.concatenate(outs, axis=1).astype(np.float32)
